# revision 10
# baseline (speedup 1.0000x reference)
"""CRF partition-function kernel for Trainium2 (8 NeuronCores).

Strategy (sequence-parallel log-semiring matrix scan):
  logZ = lse(alpha_{T-1}),  alpha_i[j] = emit_i[j] + lse_k(trans[k,j] + alpha_{i-1}[k])
is an associative chain of log-semiring matrix products with elementary
factors A_t[k,j] = trans[k,j] + emit[t,j].  T=8192 is split into 32 chunks
of L=256; each core scans 4 chunks SIMULTANEOUSLY (batched as extra moving
columns, in two alternating 2-chunk groups so each group's epilogue
overlaps the other group's matmuls) in normalized exp space: the fixed weight
E = exp(trans - c0) lives on the TensorEngine (bf16), each step is 8
matmuls [128,128]x[128,512] with fp32 PSUM accumulation, followed by a
per-row (j) scale by exp(emit_t[j]) done as two broadcast tensor_tensor
muls on VectorE (j-tile 0) and scalar-scaled copies split 3:1 between
ScalarE and VectorE (j-tile 1).  Every 64 steps a renormalizer 1/S is folded into a
future step's emission columns (exact log accounting into `acc`).
Chunk 0's first factor bakes in BOS.  The 32 chunk matrices are combined
in float64 on the host (~1 MFLOP) together with the gold score.
"""

import numpy as np
import ml_dtypes

import concourse.bass as bass
import concourse.bacc as bacc
import concourse.mybir as mybir
import concourse.tile as tile
from concourse.bass_utils import run_bass_kernel_spmd

BF16 = ml_dtypes.bfloat16
NT = 256
T_FULL = 8192
N_CORES = 8
P = 128
CPC = 4          # chunks per core
W = CPC * NT     # 1024: per-core rhs width / emission columns

_CACHE = {}


def build_nc(L, R=64, nonce=""):
    """Per-core program: scan CPC chunks of length L in lockstep.

    `nonce` only renames one DRAM tensor (forces a fresh NEFF compile for
    profiling runs without changing the program)."""
    f32 = mybir.dt.float32
    bf16 = mybir.dt.bfloat16
    Exp = mybir.ActivationFunctionType.Exp
    Ln = mybir.ActivationFunctionType.Ln
    Copy = mybir.ActivationFunctionType.Copy
    X = mybir.AxisListType.X
    ADD = mybir.AluOpType.add
    W_ = CPC * L   # per-core emission columns
    KW = CPC * NT  # per-core k-columns (chunk-batched rhs width)
    HK = 2 * NT    # k-columns per n-half (2 chunks)

    nc = bacc.Bacc(None, target_bir_lowering=False)
    emit_t = nc.declare_dram_parameter("emit_t", [NT, W_], f32, isOutput=False)
    eh = nc.declare_dram_parameter("eh", [NT, NT], bf16, isOutput=False)
    q0 = nc.declare_dram_parameter("q0" + nonce, [NT, KW], bf16, isOutput=False)
    qout = nc.declare_dram_parameter("qout", [NT, KW], bf16, isOutput=True)
    accout = nc.declare_dram_parameter("accout", [1, 1], f32, isOutput=True)

    with tile.TileContext(nc) as tc:
        with (
            tc.tile_pool(name="const", bufs=1) as cp,
            tc.tile_pool(name="state", bufs=1) as sp,
            tc.tile_pool(name="pj0", bufs=2, space=bass.MemorySpace.PSUM) as pp0,
            tc.tile_pool(name="pj1a", bufs=2, space=bass.MemorySpace.PSUM) as pp1a,
            tc.tile_pool(name="pj1b", bufs=1, space=bass.MemorySpace.PSUM) as pp1b,
            tc.tile_pool(name="psm", bufs=1, space=bass.MemorySpace.PSUM) as pq,
            tc.tile_pool(name="small", bufs=2) as mp,
        ):
            # --- constants ---
            E0 = cp.tile([P, NT], bf16, tag="E0", name="E0")  # E[m 0:128, j]
            E1 = cp.tile([P, NT], bf16, tag="E1", name="E1")  # E[m 128:256, j]
            nc.sync.dma_start(E0[:], eh[0:P, :])
            nc.sync.dma_start(E1[:], eh[P:NT, :])

            eml0 = cp.tile([P, W_], f32, tag="eml0", name="eml0")
            eml1 = cp.tile([P, W_], f32, tag="eml1", name="eml1")
            nc.sync.dma_start(eml0[:], emit_t[0:P, :])
            nc.sync.dma_start(eml1[:], emit_t[P:NT, :])
            eme0 = cp.tile([P, W_], f32, tag="eme0", name="eme0")  # exp(emit), j 0:128
            eme1 = cp.tile([P, W_], f32, tag="eme1", name="eme1")  # j 128:256
            nc.scalar.activation(eme0[:], eml0[:], Exp)
            nc.scalar.activation(eme1[:], eml1[:], Exp)

            ones_col = cp.tile([P, 1], f32, tag="ones_col", name="ones_col")
            nc.vector.memset(ones_col[:], 1.0)
            ones_row = cp.tile([1, P], f32, tag="ones_row", name="ones_row")
            nc.vector.memset(ones_row[:], 1.0)

            acc = sp.tile([1, 1], f32, tag="acc", name="acc")
            nc.vector.memset(acc[:], 0.0)

            # --- state: per-group ping-pong Q: qs[g][ph][m] = [128, HK] ---
            # group g owns chunks {2g, 2g+1}; groups advance in alternating
            # slots so a group's epilogue has the other group's slot to finish.
            qs = [
                [[sp.tile([P, HK], bf16, tag=f"q{g}{ph}{m}", name=f"q{g}{ph}{m}")
                  for m in range(2)] for ph in range(2)]
                for g in range(2)
            ]
            nc.sync.dma_start(qs[0][0][0][:], q0[0:P, 0:HK])
            nc.sync.dma_start(qs[1][0][0][:], q0[0:P, HK:KW])
            nc.sync.dma_start(qs[0][0][1][:], q0[P:NT, 0:HK])
            nc.sync.dma_start(qs[1][0][1][:], q0[P:NT, HK:KW])

            for i in range(1, L):
                ps = {}
                for g in range(2):
                    qa = qs[g][(i - 1) % 2]
                    qb = qs[g][i % 2]
                    for j in range(2):
                        pool = pp0 if j == 0 else (pp1a if g == 0 else pp1b)
                        t = pool.tile([P, HK], f32, tag=f"ps{j}{g}",
                                      name=f"ps{j}{g}")
                        ps[(j, g)] = t
                        jb = slice(j * P, (j + 1) * P)
                        nc.tensor.matmul(t[:], E0[:, jb], qa[0][:],
                                         start=True, stop=False)
                        nc.tensor.matmul(t[:], E1[:, jb], qa[1][:],
                                         start=False, stop=True)
                    # j0 on VectorE as broadcast tensor_tensor
                    src = ps[(0, g)][:].rearrange("p (g c) -> p g c", g=2)
                    base = (2 * g) * L + i
                    sc = eme0[:, base : base + L + 1 : L].broadcast_to([P, 2, NT])
                    dst = qb[0][:].rearrange("p (g c) -> p g c", g=2)
                    nc.vector.tensor_mul(dst, src, sc)
                    # j1: per-chunk scaled copies; chunk 3 goes to VectorE on
                    # 4 of 5 steps (rate-balanced split: DVE ~0.54 FD/ns on
                    # tensor_scalar vs ACT ~0.45 on scaled ACTIVATE)
                    for h in range(2):
                        cc = 2 * g + h
                        dst = qb[1][:, h * NT : (h + 1) * NT]
                        srcp = ps[(1, g)][:, h * NT : (h + 1) * NT]
                        scl = eme1[:, cc * L + i : cc * L + i + 1]
                        if g == 1 and h == 1 and i % 5 != 0:
                            nc.vector.tensor_scalar_mul(dst, srcp, scl)
                        else:
                            nc.scalar.activation(dst, srcp, Copy, scale=scl)

                if i % R == R - 1 and i + 2 < L:
                    # renorm: sample region (j0,n0); fold 1/S into step-(i+2)
                    # emission columns; exact accounting in acc.
                    rs = mp.tile([P, 1], f32, tag="rs", name="rs")
                    nc.vector.tensor_reduce(rs[:], ps[(0, 0)][:], axis=X, op=ADD)  # sample group 0
                    psS = pq.tile([1, 1], f32, tag="psm", name="psS")
                    nc.tensor.matmul(psS[:], ones_col[:], rs[:],
                                     start=True, stop=True)
                    lnS = mp.tile([1, 1], f32, tag="lnS", name="lnS")
                    nc.scalar.activation(lnS[:], psS[:], Ln)
                    nc.vector.tensor_add(acc[:], acc[:], lnS[:])
                    rin = mp.tile([1, 1], f32, tag="rin", name="rin")
                    nc.vector.reciprocal(rin[:], psS[:])
                    psB = pq.tile([P, 1], f32, tag="psm", name="psB")
                    nc.tensor.matmul(psB[:], ones_row[:], rin[:],
                                     start=True, stop=True)
                    rb = mp.tile([P, 1], f32, tag="rb", name="rb")
                    nc.vector.tensor_copy(rb[:], psB[:])
                    for eme in (eme0, eme1):
                        v = eme[:, i + 2 : i + 2 + (CPC - 1) * L + 1 : L]
                        nc.vector.tensor_scalar_mul(v, v, rb[:])

            ph = (L - 1) % 2
            nc.sync.dma_start(qout[0:P, 0:HK], qs[0][ph][0][:])
            nc.sync.dma_start(qout[0:P, HK:KW], qs[1][ph][0][:])
            nc.sync.dma_start(qout[P:NT, 0:HK], qs[0][ph][1][:])
            nc.sync.dma_start(qout[P:NT, HK:KW], qs[1][ph][1][:])
            nc.sync.dma_start(accout[:], acc[:])

    nc.compile()
    return nc


def _get_nc(L, R=64, nonce=""):
    key = (L, R, nonce)
    if key not in _CACHE:
        _CACHE[key] = build_nc(L, R, nonce)
    return _CACHE[key]


def host_prep(emit, trans, BOS, L):
    """Per-core input maps; core c owns chunks 4c..4c+3 (length L each)."""
    T = emit.shape[0]
    c0 = float(np.log(np.exp(trans.astype(np.float64)).sum(0).mean()))
    eh = np.exp(trans.astype(np.float64) - c0).astype(BF16)
    emit_t = np.ascontiguousarray(emit.T.astype(np.float32))
    n_cores = T // (CPC * L)
    in_maps = []
    for c in range(n_cores):
        base = c * CPC * L
        q0_blocks = []
        for cc in range(CPC):
            g = c * CPC + cc
            t0 = g * L
            if g == 0:
                col = np.exp(BOS.astype(np.float64) - c0
                             + emit[0].astype(np.float64))
                q0_blocks.append(np.broadcast_to(col[:, None], (NT, NT)))
            else:
                q0_blocks.append(
                    np.exp(trans.T.astype(np.float64) - c0
                           + emit[t0].astype(np.float64)[:, None])
                )
        in_maps.append(
            {
                "emit_t": np.ascontiguousarray(emit_t[:, base : base + CPC * L]),
                "eh": eh,
                "q0": np.concatenate(q0_blocks, axis=1).astype(BF16),
            }
        )
    return in_maps, c0


def host_combine(results, c0, L):
    """Fold the 32 chunk matrices into logZ (float64)."""
    a = None
    for c, r in enumerate(results):
        q = r["qout"].astype(np.float64)
        accv = float(np.asarray(r["accout"]).reshape(-1)[0])
        for cc in range(CPC):
            with np.errstate(divide="ignore"):
                G = np.log(q[:, cc * NT : (cc + 1) * NT]) + accv + L * c0
            if a is None:
                a = G[:, 0]
            else:
                z = G + a[None, :]
                m = z.max()
                a = np.log(np.exp(z - m).sum(1)) + m
    m = a.max()
    return float(np.log(np.exp(a - m).sum()) + m)


def gold_score(emit, y, trans, BOS, EOS):
    e = emit.astype(np.float64)
    t = trans.astype(np.float64)
    yy = np.asarray(y).astype(np.int64)
    T = e.shape[0]
    s = float(BOS[yy[0]])
    s += t[yy[:-1], yy[1:]].sum()
    s += e[np.arange(T - 1), yy[:-1]].sum()
    s += float(EOS[yy[-1]]) + e[T - 1, yy[-1]]
    return s


def kernel(emit, y, trans, BOS, EOS):
    emit = np.asarray(emit)
    trans = np.asarray(trans)
    BOS = np.asarray(BOS)
    EOS = np.asarray(EOS)
    L = T_FULL // (N_CORES * CPC)
    nc = _get_nc(L)
    in_maps, c0 = host_prep(emit, trans, BOS, L)
    results = run_bass_kernel_spmd(nc, in_maps, list(range(N_CORES))).results
    logZ = host_combine(results, c0, L)
    gold = gold_score(emit, y, trans, BOS, EOS)
    return np.array(np.float32(logZ - gold))
